# revision 19
# baseline (speedup 1.0000x reference)
"""Trainium2 Bass kernel for dot-product attention over a long sequence.

reference:
    scores = encoder_outputs[L, H] @ hidden[H]   (L = 262144, H = 512, f32)
    attn   = softmax(scores)[None, :]            -> [1, L]

Strategy (memory-bound problem, 512 MB of encoder_outputs reads):
  - Shard L across 8 NeuronCores (32768 rows / 64 MB per core).
  - Per core: big contiguous DMAs of E into SBUF with layout
    [128 partitions, ROWS_PER_DMA rows * 512] where partition p holds rows
    l_local = p*256 + j.  A fused DVE tensor_tensor_reduce (mult + row-sum)
    turns each [128, 512] row-block into one score column -> scores[128, 256].
  - Distributed softmax: local max (DVE reduce + gpsimd partition_all_reduce),
    fused exp+sum on the scalar engine, AllGather of the 8 (lmax, sumexp)
    pairs, closed-form combine, final tensor_scalar rescale, DMA out.
"""

import os
import sys

import numpy as np

for _p in ("/opt/trn_rl_repo",):
    if _p not in sys.path and os.path.isdir(_p):
        sys.path.insert(0, _p)

L = 262144
H = 512
NCORES = 8
L_LOCAL = L // NCORES  # 32768
P = 128
JCOLS = L_LOCAL // P  # 256 score columns per core
ROWS_PER_DMA = 16  # row-blocks (score columns) loaded per DMA
FREE = ROWS_PER_DMA * H  # 8192 f32 per partition per DMA tile (4 MB total)
NTILES = JCOLS // ROWS_PER_DMA  # 16 DMA tiles per core

_CACHE = {}


def _build_module(
    l_local=L_LOCAL,
    rows_per_dma=ROWS_PER_DMA,
    big_bufs=3,
    dma_split=("sync",),
    gp8=0,  # of every 8 row-blocks, this many mults go to gpsimd (rest DVE)
    dv8=8,  # of every 8 row-blocks, this many reduces stay on DVE (rest ACT)
    stt=True,  # fused scalar_tensor_tensor (mult+accum in one DVE op)
):
    """Build + compile the SPMD Bass module (same program on all 8 cores)."""
    from concourse import bacc, bass, bass_isa, mybir, tile

    f32 = mybir.dt.float32
    Alu = mybir.AluOpType
    Act = mybir.ActivationFunctionType

    jcols = l_local // P
    free = rows_per_dma * H
    ntiles = jcols // rows_per_dma
    assert jcols * P == l_local and ntiles * rows_per_dma == jcols

    nc = bacc.Bacc(
        "TRN2",
        target_bir_lowering=False,
        debug=False,
        num_devices=NCORES,
    )

    enc = nc.dram_tensor("enc", [l_local, H], f32, kind="ExternalInput")
    hid = nc.dram_tensor("hidden", [H], f32, kind="ExternalInput")
    attn = nc.dram_tensor("attn", [P, jcols], f32, kind="ExternalOutput")

    with tile.TileContext(nc) as tc:
        with (
            tc.tile_pool(name="io", bufs=1) as io_pool,
            tc.tile_pool(name="big", bufs=big_bufs) as big_pool,
            tc.tile_pool(name="scratch", bufs=2) as sc_pool,
            tc.tile_pool(name="dram", bufs=1, space="DRAM") as dram_pool,
        ):
            # Broadcast hidden to all 128 partitions: hb[p, :] = hidden.
            # (gpsimd DMA path keeps the sync queue free for the big loads.)
            hrow = io_pool.tile([1, H], f32)
            nc.gpsimd.dma_start(out=hrow[:, :], in_=hid.ap().unsqueeze(0))
            hb = io_pool.tile([P, H], f32)
            nc.gpsimd.partition_broadcast(hb[:, :], hrow[:, :])

            # scores[p, j] = dot(E[p*jcols + j, :], hidden)
            scores = io_pool.tile([P, jcols], f32)
            # 0-stride dummy destination for the ACT reduce's main output.
            dummy = io_pool.tile([P, 1], f32)

            # E rows viewed as [p, j, h]; l_local = p*jcols + j.
            ev = enc.ap().rearrange("(p j) h -> p j h", p=P)

            dma_engines = [getattr(nc, n) for n in dma_split]
            for t in range(ntiles):
                et = big_pool.tile([P, rows_per_dma, H], f32, name="et")
                dma_engines[t % len(dma_engines)].dma_start(
                    out=et[:, :, :],
                    in_=ev[:, t * rows_per_dma : (t + 1) * rows_per_dma, :],
                )
                for i in range(rows_per_dma):
                    col = t * rows_per_dma + i
                    if stt and ((col + 3) % 8) < dv8:
                        # fused: scores[:, col] = sum(E_block * hb); the main
                        # output is discarded into a 0-stride dummy to keep
                        # SBUF write-bank pressure off the DMA stream.
                        nc.vector.scalar_tensor_tensor(
                            out=dummy[:, :].broadcast_to((P, H)),
                            in0=et[:, i, :],
                            scalar=1.0,
                            in1=hb[:, :],
                            op0=Alu.mult,
                            op1=Alu.mult,
                            accum_out=scores[:, col : col + 1],
                        )
                        continue
                    prod = sc_pool.tile([P, H], f32, name="prod")
                    # prod = E_block * hb  (split between gpsimd and DVE)
                    if (col % 8) < gp8:
                        nc.gpsimd.tensor_tensor(
                            prod[:, :], et[:, i, :], hb[:, :], op=Alu.mult
                        )
                    else:
                        nc.vector.tensor_tensor(
                            prod[:, :], et[:, i, :], hb[:, :], op=Alu.mult
                        )
                    # scores[:, col] = sum(prod) (split between DVE and ACT)
                    if ((col + 3) % 8) < dv8:
                        outsc = sc_pool.tile([P, H], f32, name="outsc")
                        nc.vector.tensor_scalar(
                            out=outsc[:, :],
                            in0=prod[:, :],
                            scalar1=1.0,
                            scalar2=None,
                            op0=Alu.mult,
                            op1=Alu.add,
                            accum_out=scores[:, col : col + 1],
                        )
                    else:
                        nc.scalar.activation(
                            dummy[:, :].broadcast_to((P, H)),
                            prod[:, :],
                            Act.Copy,
                            bias=0.0,
                            scale=1.0,
                            accum_out=scores[:, col : col + 1],
                        )

            # Pre-warm ncfw + absorb cross-core skew: a dummy AllGather that
            # depends on a late score column, so it runs near the end of the
            # main loop, overlapped with remaining compute.
            # gpsimd DMA path: the sync queue is busy issuing the big loads
            # in-order, which would delay these far past their data dependency.
            warm_cols = sorted({jcols // 2, max(0, jcols - max(1, jcols // 16))})
            for warm_col in warm_cols:
                warm_in = dram_pool.tile([1, 1], f32, name=f"warm_in_{warm_col}")
                warm_out = dram_pool.tile(
                    [NCORES, 1], f32, addr_space="Shared", name=f"warm_out_{warm_col}"
                )
                nc.gpsimd.dma_start(
                    out=warm_in[:, :], in_=scores[0:1, warm_col : warm_col + 1]
                )
                nc.gpsimd.collective_compute(
                    "AllGather",
                    Alu.bypass,
                    replica_groups=[list(range(NCORES))],
                    ins=[warm_in.opt()],
                    outs=[warm_out.opt()],
                )

            # ---- distributed softmax ----
            stats = io_pool.tile([P, 2], f32)  # [:,0] = local max, [:,1] = local sumexp
            m1 = io_pool.tile([P, 1], f32)
            nc.vector.reduce_max(m1[:, :], scores[:, :], axis=mybir.AxisListType.X)
            nc.gpsimd.partition_all_reduce(
                stats[:, 0:1], m1[:, :], channels=P, reduce_op=bass_isa.ReduceOp.max
            )

            negl = io_pool.tile([P, 1], f32)
            nc.vector.tensor_scalar_mul(negl[:, :], stats[:, 0:1], -1.0)

            # e = exp(scores - lmax); ls = rowwise sum(e)
            e_sb = io_pool.tile([P, jcols], f32)
            ls = io_pool.tile([P, 1], f32)
            nc.scalar.activation(
                e_sb[:, :],
                scores[:, :],
                Act.Exp,
                bias=negl[:, :],
                scale=1.0,
                accum_out=ls[:, :],
            )
            nc.gpsimd.partition_all_reduce(
                stats[:, 1:2], ls[:, :], channels=P, reduce_op=bass_isa.ReduceOp.add
            )

            # AllGather the 8 (lmax, sumexp) pairs.
            cc_in = dram_pool.tile([1, 2], f32)
            cc_out = dram_pool.tile([NCORES, 2], f32, addr_space="Shared")
            nc.sync.dma_start(out=cc_in[:, :], in_=stats[0:1, :])
            nc.gpsimd.collective_compute(
                "AllGather",
                Alu.bypass,
                replica_groups=[list(range(NCORES))],
                ins=[cc_in.opt()],
                outs=[cc_out.opt()],
            )
            grow = io_pool.tile([1, 2 * NCORES], f32)
            nc.sync.dma_start(
                out=grow[:, :], in_=cc_out[:, :].rearrange("c t -> (c t)").unsqueeze(0)
            )
            gath = io_pool.tile([P, 2 * NCORES], f32)
            nc.gpsimd.partition_broadcast(gath[:, :], grow[:, :])

            # gath viewed as [P, 2, 8]: row 0 = the 8 lmax values, row 1 = sums.
            gv = gath[:, :].rearrange("p (c t) -> p t c", t=2)
            lmax_vec = gv[:, 0, :]  # [P, 8], stride 2
            lsum_vec = gv[:, 1, :]  # [P, 8], stride 2

            gmax = io_pool.tile([P, 1], f32)
            nc.vector.reduce_max(gmax[:, :], lmax_vec, axis=mybir.AxisListType.X)

            d = io_pool.tile([P, NCORES], f32)
            nc.vector.tensor_scalar_sub(d[:, :], lmax_vec, gmax[:, :])
            ed = io_pool.tile([P, NCORES], f32)
            nc.scalar.activation(ed[:, :], d[:, :], Act.Exp)

            # gsum = sum_c lsum_c * exp(lmax_c - gmax)
            prod8 = io_pool.tile([P, NCORES], f32)
            gsum = io_pool.tile([P, 1], f32)
            nc.vector.tensor_tensor(prod8[:, :], ed[:, :], lsum_vec, op=Alu.mult)
            nc.vector.reduce_sum(gsum[:, :], prod8[:, :], axis=mybir.AxisListType.X)
            inv = io_pool.tile([P, 1], f32)
            nc.vector.reciprocal(inv[:, :], gsum[:, :])

            # factor = exp(lmax - gmax) / gsum  (lmax = this core's local max)
            myd = io_pool.tile([P, 1], f32)
            nc.vector.tensor_scalar_sub(myd[:, :], stats[:, 0:1], gmax[:, :])
            myed = io_pool.tile([P, 1], f32)
            nc.scalar.activation(myed[:, :], myd[:, :], Act.Exp)
            factor = io_pool.tile([P, 1], f32)
            nc.vector.tensor_mul(factor[:, :], myed[:, :], inv[:, :])

            out_sb = io_pool.tile([P, jcols], f32)
            nc.vector.tensor_scalar_mul(out_sb[:, :], e_sb[:, :], factor[:, :])
            nc.sync.dma_start(out=attn.ap(), in_=out_sb[:, :])

    nc.compile()
    return nc


def get_module(
    l_local=L_LOCAL,
    rows_per_dma=ROWS_PER_DMA,
    big_bufs=4,
    dma_split=("sync",),
    gp8=0,
    dv8=8,
    stt=True,
):
    key = (l_local, rows_per_dma, big_bufs, dma_split, gp8, dv8, stt)
    if key not in _CACHE:
        _CACHE[key] = _build_module(
            l_local, rows_per_dma, big_bufs, dma_split, gp8, dv8, stt
        )
    return _CACHE[key]


def make_in_maps(hidden, encoder_outputs, l_local=L_LOCAL):
    hidden = np.ascontiguousarray(np.asarray(hidden), dtype=np.float32)
    enc = np.ascontiguousarray(np.asarray(encoder_outputs), dtype=np.float32)
    return [
        {"hidden": hidden, "enc": enc[c * l_local : (c + 1) * l_local]}
        for c in range(NCORES)
    ]


def gather_output(results):
    return np.concatenate([r["attn"].reshape(-1) for r in results])[None, :]


def kernel(hidden, encoder_outputs, **run_kwargs):
    from concourse import bass_utils

    nc = get_module()
    in_maps = make_in_maps(hidden, encoder_outputs)
    res = bass_utils.run_bass_kernel_spmd(
        nc, in_maps, core_ids=list(range(NCORES)), **run_kwargs
    )
    out = gather_output(res.results)
    if run_kwargs.get("trace"):
        return out, res
    return out


# revision 21
# speedup vs baseline: 1.0111x; 1.0111x over previous
"""Trainium2 Bass kernel for dot-product attention over a long sequence.

reference:
    scores = encoder_outputs[L, H] @ hidden[H]   (L = 262144, H = 512, f32)
    attn   = softmax(scores)[None, :]            -> [1, L]

Strategy (memory-bound problem, 512 MB of encoder_outputs reads):
  - Shard L across 8 NeuronCores (32768 rows / 64 MB per core).
  - Per core: big contiguous DMAs of E into SBUF with layout
    [128 partitions, ROWS_PER_DMA rows * 512] where partition p holds rows
    l_local = p*256 + j.  A fused DVE tensor_tensor_reduce (mult + row-sum)
    turns each [128, 512] row-block into one score column -> scores[128, 256].
  - Distributed softmax: local max (DVE reduce + gpsimd partition_all_reduce),
    fused exp+sum on the scalar engine, AllGather of the 8 (lmax, sumexp)
    pairs, closed-form combine, final tensor_scalar rescale, DMA out.
"""

import os
import sys

import numpy as np

for _p in ("/opt/trn_rl_repo",):
    if _p not in sys.path and os.path.isdir(_p):
        sys.path.insert(0, _p)

L = 262144
H = 512
NCORES = 8
L_LOCAL = L // NCORES  # 32768
P = 128
JCOLS = L_LOCAL // P  # 256 score columns per core
ROWS_PER_DMA = 16  # row-blocks (score columns) loaded per DMA
FREE = ROWS_PER_DMA * H  # 8192 f32 per partition per DMA tile (4 MB total)
NTILES = JCOLS // ROWS_PER_DMA  # 16 DMA tiles per core

_CACHE = {}


def _build_module(
    l_local=L_LOCAL,
    rows_per_dma=ROWS_PER_DMA,
    big_bufs=3,
    dma_split=("sync",),
    gp8=0,  # of every 8 row-blocks, this many mults go to gpsimd (rest DVE)
    dv8=8,  # of every 8 row-blocks, this many reduces stay on DVE (rest ACT)
    stt=True,  # fused scalar_tensor_tensor (mult+accum in one DVE op)
):
    """Build + compile the SPMD Bass module (same program on all 8 cores)."""
    from concourse import bacc, bass, bass_isa, mybir, tile

    f32 = mybir.dt.float32
    Alu = mybir.AluOpType
    Act = mybir.ActivationFunctionType

    jcols = l_local // P
    free = rows_per_dma * H
    ntiles = jcols // rows_per_dma
    assert jcols * P == l_local and ntiles * rows_per_dma == jcols

    nc = bacc.Bacc(
        "TRN2",
        target_bir_lowering=False,
        debug=False,
        num_devices=NCORES,
    )

    enc = nc.dram_tensor("enc", [l_local, H], f32, kind="ExternalInput")
    hid = nc.dram_tensor("hidden", [H], f32, kind="ExternalInput")
    attn = nc.dram_tensor("attn", [P, jcols], f32, kind="ExternalOutput")

    with tile.TileContext(nc) as tc:
        with (
            tc.tile_pool(name="io", bufs=1) as io_pool,
            tc.tile_pool(name="big", bufs=big_bufs) as big_pool,
            tc.tile_pool(name="scratch", bufs=2) as sc_pool,
            tc.tile_pool(name="dram", bufs=1, space="DRAM") as dram_pool,
        ):
            # Broadcast hidden to all 128 partitions: hb[p, :] = hidden.
            hrow = io_pool.tile([1, H], f32)
            nc.sync.dma_start(out=hrow[:, :], in_=hid.ap().unsqueeze(0))
            hb = io_pool.tile([P, H], f32)
            nc.gpsimd.partition_broadcast(hb[:, :], hrow[:, :])

            # scores[p, j] = dot(E[p*jcols + j, :], hidden)
            scores = io_pool.tile([P, jcols], f32)
            # 0-stride dummy destination for the ACT reduce's main output.
            dummy = io_pool.tile([P, 1], f32)

            # E rows viewed as [p, j, h]; l_local = p*jcols + j.
            ev = enc.ap().rearrange("(p j) h -> p j h", p=P)

            dma_engines = [getattr(nc, n) for n in dma_split]
            for t in range(ntiles):
                et = big_pool.tile([P, rows_per_dma, H], f32, name="et")
                dma_engines[t % len(dma_engines)].dma_start(
                    out=et[:, :, :],
                    in_=ev[:, t * rows_per_dma : (t + 1) * rows_per_dma, :],
                )
                for i in range(rows_per_dma):
                    col = t * rows_per_dma + i
                    if stt and ((col + 3) % 8) < dv8:
                        # fused: scores[:, col] = sum(E_block * hb); the main
                        # output is discarded into a 0-stride dummy to keep
                        # SBUF write-bank pressure off the DMA stream.
                        nc.vector.scalar_tensor_tensor(
                            out=dummy[:, :].broadcast_to((P, H)),
                            in0=et[:, i, :],
                            scalar=1.0,
                            in1=hb[:, :],
                            op0=Alu.mult,
                            op1=Alu.mult,
                            accum_out=scores[:, col : col + 1],
                        )
                        continue
                    prod = sc_pool.tile([P, H], f32, name="prod")
                    # prod = E_block * hb  (split between gpsimd and DVE)
                    if (col % 8) < gp8:
                        nc.gpsimd.tensor_tensor(
                            prod[:, :], et[:, i, :], hb[:, :], op=Alu.mult
                        )
                    else:
                        nc.vector.tensor_tensor(
                            prod[:, :], et[:, i, :], hb[:, :], op=Alu.mult
                        )
                    # scores[:, col] = sum(prod) (split between DVE and ACT)
                    if ((col + 3) % 8) < dv8:
                        outsc = sc_pool.tile([P, H], f32, name="outsc")
                        nc.vector.tensor_scalar(
                            out=outsc[:, :],
                            in0=prod[:, :],
                            scalar1=1.0,
                            scalar2=None,
                            op0=Alu.mult,
                            op1=Alu.add,
                            accum_out=scores[:, col : col + 1],
                        )
                    else:
                        nc.scalar.activation(
                            dummy[:, :].broadcast_to((P, H)),
                            prod[:, :],
                            Act.Copy,
                            bias=0.0,
                            scale=1.0,
                            accum_out=scores[:, col : col + 1],
                        )

            # Pre-warm ncfw + absorb cross-core skew: a dummy AllGather that
            # depends on a late score column, so it runs near the end of the
            # main loop, overlapped with remaining compute.
            # gpsimd DMA path: the sync queue is busy issuing the big loads
            # in-order, which would delay these far past their data dependency.
            warm_cols = [jcols // 2]
            for warm_col in warm_cols:
                warm_in = dram_pool.tile([1, 1], f32, name=f"warm_in_{warm_col}")
                warm_out = dram_pool.tile(
                    [NCORES, 1], f32, addr_space="Shared", name=f"warm_out_{warm_col}"
                )
                nc.gpsimd.dma_start(
                    out=warm_in[:, :], in_=scores[0:1, warm_col : warm_col + 1]
                )
                nc.gpsimd.collective_compute(
                    "AllGather",
                    Alu.bypass,
                    replica_groups=[list(range(NCORES))],
                    ins=[warm_in.opt()],
                    outs=[warm_out.opt()],
                )

            # ---- distributed softmax ----
            stats = io_pool.tile([P, 2], f32)  # [:,0] = local max, [:,1] = local sumexp
            m1 = io_pool.tile([P, 1], f32)
            nc.vector.reduce_max(m1[:, :], scores[:, :], axis=mybir.AxisListType.X)
            nc.gpsimd.partition_all_reduce(
                stats[:, 0:1], m1[:, :], channels=P, reduce_op=bass_isa.ReduceOp.max
            )

            negl = io_pool.tile([P, 1], f32)
            nc.vector.tensor_scalar_mul(negl[:, :], stats[:, 0:1], -1.0)

            # e = exp(scores - lmax); ls = rowwise sum(e)
            e_sb = io_pool.tile([P, jcols], f32)
            ls = io_pool.tile([P, 1], f32)
            nc.scalar.activation(
                e_sb[:, :],
                scores[:, :],
                Act.Exp,
                bias=negl[:, :],
                scale=1.0,
                accum_out=ls[:, :],
            )
            nc.gpsimd.partition_all_reduce(
                stats[:, 1:2], ls[:, :], channels=P, reduce_op=bass_isa.ReduceOp.add
            )

            # AllGather the 8 (lmax, sumexp) pairs.
            cc_in = dram_pool.tile([1, 2], f32)
            cc_out = dram_pool.tile([NCORES, 2], f32, addr_space="Shared")
            nc.sync.dma_start(out=cc_in[:, :], in_=stats[0:1, :])
            nc.gpsimd.collective_compute(
                "AllGather",
                Alu.bypass,
                replica_groups=[list(range(NCORES))],
                ins=[cc_in.opt()],
                outs=[cc_out.opt()],
            )
            grow = io_pool.tile([1, 2 * NCORES], f32)
            nc.sync.dma_start(
                out=grow[:, :], in_=cc_out[:, :].rearrange("c t -> (c t)").unsqueeze(0)
            )
            gath = io_pool.tile([P, 2 * NCORES], f32)
            nc.gpsimd.partition_broadcast(gath[:, :], grow[:, :])

            # gath viewed as [P, 2, 8]: row 0 = the 8 lmax values, row 1 = sums.
            gv = gath[:, :].rearrange("p (c t) -> p t c", t=2)
            lmax_vec = gv[:, 0, :]  # [P, 8], stride 2
            lsum_vec = gv[:, 1, :]  # [P, 8], stride 2

            gmax = io_pool.tile([P, 1], f32)
            nc.vector.reduce_max(gmax[:, :], lmax_vec, axis=mybir.AxisListType.X)

            d = io_pool.tile([P, NCORES], f32)
            nc.vector.tensor_scalar_sub(d[:, :], lmax_vec, gmax[:, :])
            ed = io_pool.tile([P, NCORES], f32)
            nc.scalar.activation(ed[:, :], d[:, :], Act.Exp)

            # gsum = sum_c lsum_c * exp(lmax_c - gmax)
            prod8 = io_pool.tile([P, NCORES], f32)
            gsum = io_pool.tile([P, 1], f32)
            nc.vector.tensor_tensor(prod8[:, :], ed[:, :], lsum_vec, op=Alu.mult)
            nc.vector.reduce_sum(gsum[:, :], prod8[:, :], axis=mybir.AxisListType.X)
            inv = io_pool.tile([P, 1], f32)
            nc.vector.reciprocal(inv[:, :], gsum[:, :])

            # factor = exp(lmax - gmax) / gsum  (lmax = this core's local max)
            myd = io_pool.tile([P, 1], f32)
            nc.vector.tensor_scalar_sub(myd[:, :], stats[:, 0:1], gmax[:, :])
            myed = io_pool.tile([P, 1], f32)
            nc.scalar.activation(myed[:, :], myd[:, :], Act.Exp)
            factor = io_pool.tile([P, 1], f32)
            nc.vector.tensor_mul(factor[:, :], myed[:, :], inv[:, :])

            out_sb = io_pool.tile([P, jcols], f32)
            nc.vector.tensor_scalar_mul(out_sb[:, :], e_sb[:, :], factor[:, :])
            nc.sync.dma_start(out=attn.ap(), in_=out_sb[:, :])

    nc.compile()
    return nc


def get_module(
    l_local=L_LOCAL,
    rows_per_dma=ROWS_PER_DMA,
    big_bufs=4,
    dma_split=("sync",),
    gp8=0,
    dv8=8,
    stt=True,
):
    key = (l_local, rows_per_dma, big_bufs, dma_split, gp8, dv8, stt)
    if key not in _CACHE:
        _CACHE[key] = _build_module(
            l_local, rows_per_dma, big_bufs, dma_split, gp8, dv8, stt
        )
    return _CACHE[key]


def make_in_maps(hidden, encoder_outputs, l_local=L_LOCAL):
    hidden = np.ascontiguousarray(np.asarray(hidden), dtype=np.float32)
    enc = np.ascontiguousarray(np.asarray(encoder_outputs), dtype=np.float32)
    return [
        {"hidden": hidden, "enc": enc[c * l_local : (c + 1) * l_local]}
        for c in range(NCORES)
    ]


def gather_output(results):
    return np.concatenate([r["attn"].reshape(-1) for r in results])[None, :]


def kernel(hidden, encoder_outputs, **run_kwargs):
    from concourse import bass_utils

    nc = get_module()
    in_maps = make_in_maps(hidden, encoder_outputs)
    res = bass_utils.run_bass_kernel_spmd(
        nc, in_maps, core_ids=list(range(NCORES)), **run_kwargs
    )
    out = gather_output(res.results)
    if run_kwargs.get("trace"):
        return out, res
    return out
